# revision 7
# baseline (speedup 1.0000x reference)
"""MultiHeadAttention with RoPE on 8 Trainium2 NeuronCores.

Sharding: core c = (batch b = c//4, head-group hg = c%4, 4 heads/group).
Per core: bf16 QKV projections (from host-pretransposed operands), RoPE
applied via shuffle-DMA + DVE multiply-adds against host-baked cos/sin
tables (softmax scale folded into Q's tables), flash-style attention in
scores-transposed layout (keys on partitions) with a ones-column appended
to V so the PV matmul also produces softmax denominators, per-head
normalization via DMA-broadcast reciprocal, then an 8-rank AllToAll that
routes each core the attn^T slice for its q-share (the other batch's
shards are cancelled by per-core zero-padded Wo), and a q-sharded
out-projection.  Host reassembles the 8 per-core (512, 1024) outputs.
"""

import sys

sys.path.insert(0, "/opt/trn_rl_repo")

import numpy as np
import ml_dtypes

import concourse.bass as bass
import concourse.bacc as bacc
import concourse.tile as tile
from concourse import mybir
from concourse.bass_utils import run_bass_kernel_spmd

BF16 = ml_dtypes.bfloat16
F32 = mybir.dt.float32
BF = mybir.dt.bfloat16

B, S, D = 2, 2048, 1024
H, HD = 16, 64
HPC = 4              # heads per core
N_CORES = 8
QSH = S // 4         # per-core q share for out-proj (512)
NT = S // 128        # 16 key/token tiles
SCALE = HD ** -0.5


def build_nc(use_mask: bool):
    nc = bacc.Bacc("TRN2", target_bir_lowering=False, debug=False,
                   num_devices=N_CORES)

    xt_d = nc.dram_tensor("xt", [D, S], BF, kind="ExternalInput")
    wq_d = nc.dram_tensor("wqT", [D, 256], BF, kind="ExternalInput")
    wk_d = nc.dram_tensor("wkT", [D, 256], BF, kind="ExternalInput")
    wv_d = nc.dram_tensor("wvT", [D, 256], BF, kind="ExternalInput")
    wo_d = nc.dram_tensor("woT", [2 * D, D], BF, kind="ExternalInput")
    cq_d = nc.dram_tensor("ropeQcos", [128, S], BF, kind="ExternalInput")
    sq_d = nc.dram_tensor("ropeQsin", [128, S], BF, kind="ExternalInput")
    ck_d = nc.dram_tensor("ropeKcos", [128, S], BF, kind="ExternalInput")
    sk_d = nc.dram_tensor("ropeKsin", [128, S], BF, kind="ExternalInput")
    if use_mask:
        mb_d = nc.dram_tensor("maskT", [S, S], BF, kind="ExternalInput")
    out_d = nc.dram_tensor("out", [QSH, D], F32, kind="ExternalOutput")

    with tile.TileContext(nc) as tc:
        with tc.tile_pool(name="cpool", bufs=1) as cpool, \
             tc.tile_pool(name="tpool", bufs=2) as tpool, \
             tc.tile_pool(name="ppool", bufs=2, space="PSUM") as ppool, \
             tc.tile_pool(name="dpool", bufs=1, space="DRAM") as dpool:

            tabs = {}
            for nm, dram in (("cq", cq_d), ("sq", sq_d), ("ck", ck_d), ("sk", sk_d)):
                t = cpool.tile([128, S], BF, name=f"tab_{nm}", tag=f"tab_{nm}")
                nc.sync.dma_start(out=t[:], in_=dram[:, :])
                tabs[nm] = t
            # v with ones column: [128, tt, head, 65]
            vt_all = cpool.tile([128, NT, HPC, 65], BF, name="vt_all", tag="vt_all")
            nc.vector.memset(vt_all[:, :, :, 64:65], 1.0)

            qr = [cpool.tile([128, S], BF, name=f"qr{i}", tag=f"qr{i}")
                  for i in range(2)]
            kr = [cpool.tile([128, S], BF, name=f"kr{i}", tag=f"kr{i}")
                  for i in range(2)]
            at_sb = [cpool.tile([64, S], BF, name=f"at_sb{h}", tag=f"at_sb{h}")
                     for h in range(HPC)]

            a2a_in = dpool.tile([2 * D, QSH], BF, name="a2a_in")
            a2a_out = dpool.tile([2 * D, QSH], BF, name="a2a_out")

            def proj_pair(i, w_sb, dst, ctab, stab, xt_sb):
                """project hd dims [128i:128i+128] for all q, rope, into dst."""
                raw = tpool.tile([128, S], BF, name="raw", tag="rope_raw")
                for qh in range(2):
                    ps = ppool.tile([128, 1024], F32, name="proj_ps", tag="st")
                    for nck in range(2):
                        for kc in range(8):
                            nc.tensor.matmul(
                                out=ps[:, 512 * nck:512 * nck + 512],
                                lhsT=w_sb[kc][:, 128 * i:128 * i + 128],
                                rhs=xt_sb[kc][:, 1024 * qh + 512 * nck:
                                              1024 * qh + 512 * nck + 512],
                                start=(kc == 0), stop=(kc == 7))
                    nc.scalar.copy(out=raw[:, 1024 * qh:1024 * qh + 1024], in_=ps[:])
                shuf = tpool.tile([128, S], BF, name="shuf", tag="rope_shuf")
                for blk in range(4):
                    src = 32 * (blk ^ 1)
                    nc.sync.dma_start(out=shuf[32 * blk:32 * blk + 32, :],
                                      in_=raw[src:src + 32, :])
                t1 = tpool.tile([128, S], BF, name="t1", tag="rope_t1")
                t2 = tpool.tile([128, S], BF, name="t2", tag="rope_t2")
                nc.vector.tensor_mul(t1[:], raw[:], ctab[:])
                nc.vector.tensor_mul(t2[:], shuf[:], stab[:])
                nc.vector.tensor_add(dst[:], t1[:], t2[:])

            def attn_head(hh):
                i, pb = hh // 2, 64 * (hh % 2)
                at_ps = ppool.tile([65, S], F32, name="at_ps", tag="at", bufs=1)
                for tt in range(NT):
                    for qh in range(2):
                        st = ppool.tile([128, 1024], F32, name="st", tag="st")
                        for nck in range(2):
                            nc.tensor.matmul(
                                out=st[:, 512 * nck:512 * nck + 512],
                                lhsT=kr[i][pb:pb + 64, 128 * tt:128 * tt + 128],
                                rhs=qr[i][pb:pb + 64,
                                          1024 * qh + 512 * nck:
                                          1024 * qh + 512 * nck + 512],
                                start=True, stop=True)
                        if use_mask:
                            mb = tpool.tile([128, 1024], BF, name="mb", tag="mb")
                            nc.sync.dma_start(
                                out=mb[:],
                                in_=mb_d[128 * tt:128 * tt + 128,
                                         1024 * qh:1024 * qh + 1024])
                            nc.vector.tensor_add(st[:], st[:], mb[:])
                        pt = tpool.tile([128, 1024], BF, name="pt", tag="pt", bufs=3)
                        nc.scalar.activation(out=pt[:], in_=st[:],
                                             func=mybir.ActivationFunctionType.Exp)
                        for nck in range(2):
                            nc.tensor.matmul(
                                out=at_ps[:, 1024 * qh + 512 * nck:
                                          1024 * qh + 512 * nck + 512],
                                lhsT=vt_all[:, tt, hh, :],
                                rhs=pt[:, 512 * nck:512 * nck + 512],
                                start=(tt == 0), stop=(tt == NT - 1))
                # normalize: r = 1/denom, DMA-broadcast over 64 partitions
                r65 = tpool.tile([65, S], F32, name="r65", tag="r65")
                nc.vector.reciprocal(out=r65[64:65, :], in_=at_ps[64:65, :])
                r_dram = dpool.tile([1, S], F32, name="r_dram", tag="r_dram",
                                    bufs=2)
                nc.sync.dma_start(out=r_dram[:], in_=r65[64:65, :])
                rbc = tpool.tile([64, S], F32, name="rbc", tag="rbc")
                bcast_src = bass.AP(tensor=r_dram.tensor, offset=r_dram.offset,
                                    ap=[[0, 64]] + [list(p) for p in r_dram.ap[1:]])
                nc.gpsimd.dma_start(out=rbc[:], in_=bcast_src)
                nc.vector.tensor_mul(at_sb[hh][:], at_ps[0:64, :], rbc[:])

            # ---- phase 1: projections + attention ----
            with tc.tile_pool(name="xpool", bufs=1) as xpool:
                xt_sb, wq_sb, wk_sb, wv_sb = [], [], [], []
                for kc in range(8):
                    t = xpool.tile([128, S], BF, name=f"xt{kc}", tag=f"xt{kc}")
                    nc.sync.dma_start(out=t[:], in_=xt_d[128 * kc:128 * kc + 128, :])
                    xt_sb.append(t)
                    for lst, dram, nm in ((wq_sb, wq_d, "wq"),
                                          (wk_sb, wk_d, "wk"),
                                          (wv_sb, wv_d, "wv")):
                        w = xpool.tile([128, 256], BF, name=f"{nm}{kc}",
                                       tag=f"{nm}{kc}")
                        nc.sync.dma_start(out=w[:],
                                          in_=dram[128 * kc:128 * kc + 128, :])
                        lst.append(w)

                proj_pair(0, wq_sb, qr[0], tabs["cq"], tabs["sq"], xt_sb)
                proj_pair(0, wk_sb, kr[0], tabs["ck"], tabs["sk"], xt_sb)
                for tt in range(NT):
                    vp = ppool.tile([128, 256], F32, name="vp", tag="st")
                    for kc in range(8):
                        nc.tensor.matmul(out=vp[:],
                                         lhsT=xt_sb[kc][:, 128 * tt:128 * tt + 128],
                                         rhs=wv_sb[kc][:],
                                         start=(kc == 0), stop=(kc == 7))
                    nc.vector.tensor_copy(
                        out=vt_all[:, tt, :, 0:64],
                        in_=vp[:].rearrange("p (h d) -> p h d", h=HPC))

                attn_head(0)
                attn_head(1)
                proj_pair(1, wq_sb, qr[1], tabs["cq"], tabs["sq"], xt_sb)
                proj_pair(1, wk_sb, kr[1], tabs["ck"], tabs["sk"], xt_sb)
                attn_head(2)
                attn_head(3)

            # ---- phase 2: exchange + out-projection ----
            # shard j (rows 256j..256j+256) = my attnT for q-quarter j%4
            for j in range(8):
                qoff = 512 * (j % 4)
                for hh in range(HPC):
                    nc.sync.dma_start(
                        out=a2a_in[256 * j + 64 * hh:256 * j + 64 * hh + 64, :],
                        in_=at_sb[hh][:, qoff:qoff + QSH])
            nc.gpsimd.collective_compute(
                "AllToAll", mybir.AluOpType.bypass,
                replica_groups=[list(range(N_CORES))],
                ins=[a2a_in.opt()], outs=[a2a_out.opt()])

            with tc.tile_pool(name="wpool", bufs=1) as wpool:
                wo_sb, agt = [], []
                for kc in range(16):
                    w = wpool.tile([128, D], BF, name=f"wo{kc}", tag=f"wo{kc}")
                    nc.sync.dma_start(out=w[:],
                                      in_=wo_d[128 * kc:128 * kc + 128, :])
                    wo_sb.append(w)
                    a = wpool.tile([128, QSH], BF, name=f"agt{kc}", tag=f"agt{kc}")
                    nc.sync.dma_start(out=a[:],
                                      in_=a2a_out[128 * kc:128 * kc + 128, :])
                    agt.append(a)
                for qq in range(QSH // 128):
                    op = ppool.tile([128, 1024], F32, name="op", tag="st")
                    for nck in range(2):
                        for kc in range(16):
                            nc.tensor.matmul(
                                out=op[:, 512 * nck:512 * nck + 512],
                                lhsT=agt[kc][:, 128 * qq:128 * qq + 128],
                                rhs=wo_sb[kc][:, 512 * nck:512 * nck + 512],
                                start=(kc == 0), stop=(kc == 15))
                    osb = tpool.tile([128, 1024], F32, name="osb", tag="osb")
                    nc.scalar.copy(out=osb[:], in_=op[:])
                    nc.sync.dma_start(out=out_d[128 * qq:128 * qq + 128, :],
                                      in_=osb[:])

    nc.compile()
    return nc


_NC_CACHE = {}
TRACE = False
LAST_RESULT = None


def _get_nc(use_mask: bool):
    if use_mask not in _NC_CACHE:
        _NC_CACHE[use_mask] = build_nc(use_mask)
    return _NC_CACHE[use_mask]


def _rope_tables():
    inv_freq = 1.0 / (10000.0 ** (np.arange(0, HD, 2, dtype=np.float32) / HD))
    t = np.arange(S, dtype=np.float32)
    freqs = np.einsum("s,d->sd", t, inv_freq)
    emb = np.concatenate([freqs, freqs], axis=-1)        # (S, 64)
    cos64 = np.cos(emb).T.astype(np.float32)             # (64, S)
    sin64 = np.sin(emb).T.astype(np.float32)
    sign = np.where(np.arange(HD) < 32, -1.0, 1.0)[:, None].astype(np.float32)
    sinp = sign * sin64
    cosT = np.tile(cos64, (2, 1))                        # (128, S)
    sinT = np.tile(sinp, (2, 1))
    return cosT, sinT


def kernel(x, attention_mask, Wq, Wk, Wv, Wo):
    x = np.asarray(x)
    attention_mask = np.asarray(attention_mask)
    Wq, Wk, Wv, Wo = (np.asarray(w) for w in (Wq, Wk, Wv, Wo))

    mask_trivial = bool((attention_mask != 0).all())
    nc = _get_nc(not mask_trivial)

    cosT, sinT = _rope_tables()
    cq = (cosT * SCALE).astype(BF16)
    sq = (sinT * SCALE).astype(BF16)
    ck = cosT.astype(BF16)
    sk = sinT.astype(BF16)
    woT = Wo.T.astype(np.float32)                        # (1024, 1024)

    in_maps = []
    for c in range(N_CORES):
        b, hg = c // 4, c % 4
        rows = slice(256 * hg, 256 * hg + 256)
        # per-core zero-padded extended Wo: row 256*i + d is Wo.T row
        # 256*(i%4)+d if sender i is in my batch group, else 0.
        wo_ext = np.zeros((2 * D, D), dtype=BF16)
        my = slice(1024 * b, 1024 * b + 1024)
        wo_ext[my] = woT.astype(BF16)
        m = {
            "xt": np.ascontiguousarray(x[b].T).astype(BF16),
            "wqT": np.ascontiguousarray(Wq[rows, :].T).astype(BF16),
            "wkT": np.ascontiguousarray(Wk[rows, :].T).astype(BF16),
            "wvT": np.ascontiguousarray(Wv[rows, :].T).astype(BF16),
            "woT": wo_ext,
            "ropeQcos": cq, "ropeQsin": sq,
            "ropeKcos": ck, "ropeKsin": sk,
        }
        if not mask_trivial:
            bias = np.where(attention_mask[b] == 0, -30000.0, 0.0)
            m["maskT"] = np.ascontiguousarray(bias.T).astype(BF16)
        in_maps.append(m)

    global LAST_RESULT
    res = run_bass_kernel_spmd(nc, in_maps, list(range(N_CORES)),
                               trace=TRACE, trace_cores=list(range(N_CORES)))
    LAST_RESULT = res

    out = np.empty((B, S, D), dtype=np.float32)
    for c in range(N_CORES):
        b, hg = c // 4, c % 4
        out[b, QSH * hg:QSH * hg + QSH, :] = res.results[c]["out"]
    return out


# revision 16
# speedup vs baseline: 1.0922x; 1.0922x over previous
"""MultiHeadAttention with RoPE on 8 Trainium2 NeuronCores.

Sharding: core c = (batch b = c//4, head-group hg = c%4, 4 heads/group).
Per core: bf16 QKV projections (host-pretransposed operands), RoPE via
shuffle-DMA + DVE multiply-adds against host-baked cos/sin tables
(softmax scale folded into Q's tables), flash-style attention in
scores-transposed layout (keys on partitions) with a ones-column appended
to V so the PV matmul also produces softmax denominators, per-head
normalization via fast-reciprocal + DRAM-broadcast, an 8-rank AllToAll
that routes each core the attn^T slice for its q-share (other batch's
shards cancelled by per-core zero-padded Wo), and a q-sharded
out-projection.  Attention runs in two q-passes with one AllToAll per
pass so the collectives overlap compute.  Host reassembles the 8
per-core (512, 1024) outputs.
"""

import sys

sys.path.insert(0, "/opt/trn_rl_repo")

import numpy as np
import ml_dtypes

import concourse.bass as bass
import concourse.bacc as bacc
import concourse.tile as tile
from concourse import mybir
from concourse.bass_utils import run_bass_kernel_spmd

BF16 = ml_dtypes.bfloat16
F32 = mybir.dt.float32
BF = mybir.dt.bfloat16

B, S, D = 2, 2048, 1024
H, HD = 16, 64
HPC = 4              # heads per core
N_CORES = 8
QSH = S // 4         # per-core q share for out-proj (512)
NT = S // 128        # 16 key/token tiles
NP = 2               # attention q-passes (collective overlap)
PW = QSH // NP       # per-pass width within each q-quarter (256)
PQ = 4 * PW          # per-pass total q columns (1024)
SCALE = HD ** -0.5


def build_nc(use_mask: bool):
    nc = bacc.Bacc("TRN2", target_bir_lowering=False, debug=False,
                   num_devices=N_CORES)

    xt_d = nc.dram_tensor("xt", [D, S], BF, kind="ExternalInput")
    wq_d = nc.dram_tensor("wqT", [D, 256], BF, kind="ExternalInput")
    wk_d = nc.dram_tensor("wkT", [D, 256], BF, kind="ExternalInput")
    wv_d = nc.dram_tensor("wvT", [D, 256], BF, kind="ExternalInput")
    wo_d = nc.dram_tensor("woT", [2 * D, D], BF, kind="ExternalInput")
    cq_d = nc.dram_tensor("ropeQcos", [128, S], BF, kind="ExternalInput")
    sq_d = nc.dram_tensor("ropeQsin", [128, S], BF, kind="ExternalInput")
    ck_d = nc.dram_tensor("ropeKcos", [128, S], BF, kind="ExternalInput")
    sk_d = nc.dram_tensor("ropeKsin", [128, S], BF, kind="ExternalInput")
    if use_mask:
        mb_d = nc.dram_tensor("maskT", [S, S], BF, kind="ExternalInput")
    out_d = nc.dram_tensor("out", [QSH, D], F32, kind="ExternalOutput")

    with tile.TileContext(nc) as tc:
        with tc.tile_pool(name="cpool", bufs=1) as cpool, \
             tc.tile_pool(name="tpool", bufs=2) as tpool, \
             tc.tile_pool(name="ppool", bufs=2, space="PSUM") as ppool, \
             tc.tile_pool(name="dpool", bufs=1, space="DRAM") as dpool:

            def attn_head(hh, p):
                """pass p covers global q cols 512*g + PW*p + [0:PW), g=0..3.
                at_sb columns are pass-local: col = PQ*p + PW*g + j."""
                i, pb = hh // 2, 64 * (hh % 2)
                at_ps = ppool.tile([65, PQ], F32, name="at_ps", tag="at", bufs=2)
                for tt in range(NT):
                    st = ppool.tile([128, PQ], F32, name="st", tag="st", bufs=2)
                    for nck in range(2):
                        nc.tensor.matmul(
                            out=st[:, 512 * nck:512 * nck + 512],
                            lhsT=kr[i][pb:pb + 64, 128 * tt:128 * tt + 128],
                            rhs=qr[i][pb:pb + 64, PQ * p + 512 * nck:
                                      PQ * p + 512 * nck + 512],
                            start=True, stop=True)
                    if use_mask:
                        mb = tpool.tile([128, PQ], BF, name="mb", tag="mb")
                        nc.sync.dma_start(
                            out=mb[:],
                            in_=mb_d[128 * tt:128 * tt + 128,
                                     PQ * p:PQ * p + PQ])
                        nc.vector.tensor_add(st[:], st[:], mb[:])
                    pt = tpool.tile([128, PQ], BF, name="pt", tag="pt", bufs=3)
                    nc.scalar.activation(out=pt[:], in_=st[:],
                                         func=mybir.ActivationFunctionType.Exp)
                    for nck in range(2):
                        nc.tensor.matmul(
                            out=at_ps[:, 512 * nck:512 * nck + 512],
                            lhsT=vt_all[:, tt, hh, :],
                            rhs=pt[:, 512 * nck:512 * nck + 512],
                            start=(tt == 0), stop=(tt == NT - 1))
                # normalize: r = 1/denom, DRAM-broadcast over 64 partitions
                r65 = tpool.tile([65, PQ], F32, name="r65", tag="r65")
                nc.vector.reciprocal(out=r65[64:65, :], in_=at_ps[64:65, :])
                r_dram = dpool.tile([1, PQ], F32, name="r_dram", tag="r_dram",
                                    bufs=2)
                nc.sync.dma_start(out=r_dram[:], in_=r65[64:65, :])
                rbc = tpool.tile([64, PQ], F32, name="rbc", tag="rbc")
                bcast_src = bass.AP(tensor=r_dram.tensor, offset=r_dram.offset,
                                    ap=[[0, 64]] + [list(q) for q in r_dram.ap[1:]])
                nc.gpsimd.dma_start(out=rbc[:], in_=bcast_src)
                nc.vector.tensor_mul(at_sb[hh][:, PQ * p:PQ * p + PQ],
                                     at_ps[0:64, :], rbc[:])

            def exchange(p, a2a_in, a2a_out):
                for j in range(8):
                    for hh in range(HPC):
                        nc.sync.dma_start(
                            out=a2a_in[256 * j + 64 * hh:
                                       256 * j + 64 * hh + 64, :],
                            in_=at_sb[hh][:, PQ * p + PW * (j % 4):
                                          PQ * p + PW * (j % 4) + PW])
                nc.gpsimd.collective_compute(
                    "AllToAll", mybir.AluOpType.bypass,
                    replica_groups=[list(range(N_CORES))],
                    ins=[a2a_in.opt()], outs=[a2a_out.opt()])

            def outproj(p, a2a_out):
                agt = []
                for kc in range(16):
                    a = wpool.tile([128, PW], BF, name=f"agt{kc}",
                                   tag=f"agt{kc}", bufs=2)
                    nc.sync.dma_start(out=a[:],
                                      in_=a2a_out[128 * kc:128 * kc + 128, :])
                    agt.append(a)
                for qq in range(PW // 128):
                    op = ppool.tile([128, 1024], F32, name="op", tag="st", bufs=2)
                    for nck in range(2):
                        for kc in range(16):
                            nc.tensor.matmul(
                                out=op[:, 512 * nck:512 * nck + 512],
                                lhsT=agt[kc][:, 128 * qq:128 * qq + 128],
                                rhs=wo_sb[kc][:, 512 * nck:512 * nck + 512],
                                start=(kc == 0), stop=(kc == 15))
                    osb = tpool.tile([128, 1024], F32, name="osb", tag="osb")
                    nc.scalar.copy(out=osb[:], in_=op[:])
                    nc.sync.dma_start(
                        out=out_d[PW * p + 128 * qq:PW * p + 128 * qq + 128, :],
                        in_=osb[:])

            def proj_pair(i, w_sb, dst, ctab, stab, xt_sb):
                """project hd dims [128i:128i+128] for all q, rope, into dst."""
                raw = tpool.tile([128, S], BF, name="raw", tag="rope_raw")
                for qh in range(2):
                    ps = ppool.tile([128, 1024], F32, name="proj_ps", tag="st", bufs=2)
                    for nck in range(2):
                        for kc in range(8):
                            nc.tensor.matmul(
                                out=ps[:, 512 * nck:512 * nck + 512],
                                lhsT=w_sb[kc][:, 128 * i:128 * i + 128],
                                rhs=xt_sb[kc][:, 1024 * qh + 512 * nck:
                                              1024 * qh + 512 * nck + 512],
                                start=(kc == 0), stop=(kc == 7))
                    nc.scalar.copy(out=raw[:, 1024 * qh:1024 * qh + 1024], in_=ps[:])
                shuf = tpool.tile([128, S], BF, name="shuf", tag="rope_shuf")
                for blk in range(4):
                    src = 32 * (blk ^ 1)
                    nc.sync.dma_start(out=shuf[32 * blk:32 * blk + 32, :],
                                      in_=raw[src:src + 32, :])
                t1 = tpool.tile([128, S], BF, name="t1", tag="rope_t1")
                t2 = tpool.tile([128, S], BF, name="t2", tag="rope_t2")
                nc.vector.tensor_mul(t1[:], raw[:], ctab[:])
                nc.vector.tensor_mul(t2[:], shuf[:], stab[:])
                nc.vector.tensor_add(dst[:], t1[:], t2[:])

            # ---- persistent tiles ----
            vt_all = cpool.tile([128, NT, HPC, 65], BF, name="vt_all",
                                tag="vt_all")
            qr = [cpool.tile([128, S], BF, name=f"qr{i}", tag=f"qr{i}")
                  for i in range(2)]
            kr = [cpool.tile([128, S], BF, name=f"kr{i}", tag=f"kr{i}")
                  for i in range(2)]
            at_sb = [cpool.tile([64, S], BF, name=f"at_sb{h}", tag=f"at_sb{h}")
                     for h in range(HPC)]
            a2a = [(dpool.tile([2 * D, PW], BF, name=f"a2a_in{p}", tag=f"a2a_in{p}"),
                    dpool.tile([2 * D, PW], BF, name=f"a2a_out{p}", tag=f"a2a_out{p}"))
                   for p in range(NP)]

            # ---- phase 1: projections + attention pass 0 ----
            with tc.tile_pool(name="xpool", bufs=1) as xpool:
                xt_sb, wq_sb, wk_sb, wv_sb = [], [], [], []
                for kc in range(8):
                    t = xpool.tile([128, S], BF, name=f"xt{kc}", tag=f"xt{kc}")
                    nc.sync.dma_start(out=t[:], in_=xt_d[128 * kc:128 * kc + 128, :])
                    xt_sb.append(t)
                    for lst, dram, nm in ((wq_sb, wq_d, "wq"),
                                          (wk_sb, wk_d, "wk"),
                                          (wv_sb, wv_d, "wv")):
                        w = xpool.tile([128, 256], BF, name=f"{nm}{kc}",
                                       tag=f"{nm}{kc}")
                        nc.sync.dma_start(out=w[:],
                                          in_=dram[128 * kc:128 * kc + 128, :])
                        lst.append(w)
                tabs = {}
                for nm, dram in (("cq", cq_d), ("sq", sq_d),
                                 ("ck", ck_d), ("sk", sk_d)):
                    t = cpool.tile([128, S], BF, name=f"tab_{nm}", tag=f"tab_{nm}")
                    nc.sync.dma_start(out=t[:], in_=dram[:, :])
                    tabs[nm] = t
                nc.vector.memset(vt_all[:, :, :, 64:65], 1.0)

                proj_pair(0, wq_sb, qr[0], tabs["cq"], tabs["sq"], xt_sb)
                proj_pair(0, wk_sb, kr[0], tabs["ck"], tabs["sk"], xt_sb)
                for tt in range(NT):
                    vp = ppool.tile([128, 256], F32, name="vp", tag="st", bufs=2)
                    for kc in range(8):
                        nc.tensor.matmul(out=vp[:],
                                         lhsT=xt_sb[kc][:, 128 * tt:128 * tt + 128],
                                         rhs=wv_sb[kc][:],
                                         start=(kc == 0), stop=(kc == 7))
                    nc.vector.tensor_copy(
                        out=vt_all[:, tt, :, 0:64],
                        in_=vp[:].rearrange("p (h d) -> p h d", h=HPC))

                attn_head(0, 0)
                attn_head(1, 0)
                proj_pair(1, wq_sb, qr[1], tabs["cq"], tabs["sq"], xt_sb)
                proj_pair(1, wk_sb, kr[1], tabs["ck"], tabs["sk"], xt_sb)

            with tc.tile_pool(name="wpool", bufs=1) as wpool:
                wo_sb = []
                for kc in range(16):
                    w = wpool.tile([128, D], BF, name=f"wo{kc}", tag=f"wo{kc}")
                    nc.sync.dma_start(out=w[:],
                                      in_=wo_d[128 * kc:128 * kc + 128, :])
                    wo_sb.append(w)

                attn_head(2, 0)
                attn_head(3, 0)
                exchange(0, *a2a[0])

                attn_head(0, 1)
                attn_head(1, 1)
                outproj(0, a2a[0][1])
                attn_head(2, 1)
                attn_head(3, 1)
                exchange(1, *a2a[1])
                outproj(1, a2a[1][1])

    nc.compile()
    return nc


_NC_CACHE = {}
TRACE = False
LAST_RESULT = None


def _get_nc(use_mask: bool):
    if use_mask not in _NC_CACHE:
        _NC_CACHE[use_mask] = build_nc(use_mask)
    return _NC_CACHE[use_mask]


def _rope_tables():
    inv_freq = 1.0 / (10000.0 ** (np.arange(0, HD, 2, dtype=np.float32) / HD))
    t = np.arange(S, dtype=np.float32)
    freqs = np.einsum("s,d->sd", t, inv_freq)
    emb = np.concatenate([freqs, freqs], axis=-1)        # (S, 64)
    cos64 = np.cos(emb).T.astype(np.float32)             # (64, S)
    sin64 = np.sin(emb).T.astype(np.float32)
    sign = np.where(np.arange(HD) < 32, -1.0, 1.0)[:, None].astype(np.float32)
    sinp = sign * sin64
    cosT = np.tile(cos64, (2, 1))                        # (128, S)
    sinT = np.tile(sinp, (2, 1))
    return cosT, sinT


def kernel(x, attention_mask, Wq, Wk, Wv, Wo):
    x = np.asarray(x)
    attention_mask = np.asarray(attention_mask)
    Wq, Wk, Wv, Wo = (np.asarray(w) for w in (Wq, Wk, Wv, Wo))

    mask_trivial = bool((attention_mask != 0).all())
    nc = _get_nc(not mask_trivial)

    # pass-local q permutation: device col 1024p+256g+j <- global q 512g+256p+j
    perm = np.empty(S, dtype=np.int64)
    for p in range(NP):
        for g in range(4):
            j = np.arange(PW)
            perm[PQ * p + PW * g + j] = 512 * g + PW * p + j

    cosT, sinT = _rope_tables()
    cosT, sinT = cosT[:, perm], sinT[:, perm]
    cq = (cosT * SCALE).astype(BF16)
    sq = (sinT * SCALE).astype(BF16)
    ck = cosT.astype(BF16)
    sk = sinT.astype(BF16)
    woT = Wo.T.astype(np.float32)                        # (1024, 1024)

    in_maps = []
    for c in range(N_CORES):
        b, hg = c // 4, c % 4
        rows = slice(256 * hg, 256 * hg + 256)
        # per-core zero-padded extended Wo: row 256*i + d is Wo.T row
        # 256*(i%4)+d if sender i is in my batch group, else 0.
        wo_ext = np.zeros((2 * D, D), dtype=BF16)
        my = slice(1024 * b, 1024 * b + 1024)
        wo_ext[my] = woT.astype(BF16)
        m = {
            "xt": np.ascontiguousarray(x[b].T[:, perm]).astype(BF16),
            "wqT": np.ascontiguousarray(Wq[rows, :].T).astype(BF16),
            "wkT": np.ascontiguousarray(Wk[rows, :].T).astype(BF16),
            "wvT": np.ascontiguousarray(Wv[rows, :].T).astype(BF16),
            "woT": wo_ext,
            "ropeQcos": cq, "ropeQsin": sq,
            "ropeKcos": ck, "ropeKsin": sk,
        }
        if not mask_trivial:
            bias = np.where(attention_mask[b] == 0, -30000.0, 0.0)
            m["maskT"] = np.ascontiguousarray(
                bias.T[perm][:, perm]).astype(BF16)
        in_maps.append(m)

    global LAST_RESULT
    res = run_bass_kernel_spmd(nc, in_maps, list(range(N_CORES)),
                               trace=TRACE, trace_cores=list(range(N_CORES)))
    LAST_RESULT = res

    out = np.empty((B, S, D), dtype=np.float32)
    for c in range(N_CORES):
        b, hg = c // 4, c % 4
        out[b, QSH * hg:QSH * hg + QSH, :] = res.results[c]["out"]
    return out


# revision 22
# speedup vs baseline: 1.2677x; 1.1608x over previous
"""MultiHeadAttention with RoPE on 8 Trainium2 NeuronCores.

Sharding: core c = (batch b = c//4, head-group hg = c%4, 4 heads/group).
Per core: bf16 QKV projections (host-pretransposed operands), RoPE via
shuffle-DMA + DVE multiply-adds against host-baked cos/sin tables
(softmax scale folded into Q's tables), flash-style attention in
scores-transposed layout (keys on partitions) with a ones-column appended
to V so the PV matmul also produces softmax denominators, per-head
normalization via fast-reciprocal + DRAM-broadcast, an 8-rank AllToAll
that routes each core the attn^T slice for its q-share (other batch's
shards cancelled by per-core zero-padded Wo), and a q-sharded
out-projection.  Attention runs in two q-passes with one AllToAll per
pass so the collectives overlap compute.  Host reassembles the 8
per-core (512, 1024) outputs.
"""

import sys

sys.path.insert(0, "/opt/trn_rl_repo")

import numpy as np
import ml_dtypes

import concourse.bass as bass
import concourse.bacc as bacc
import concourse.tile as tile
from concourse import mybir
from concourse.bass_utils import run_bass_kernel_spmd

BF16 = ml_dtypes.bfloat16
F32 = mybir.dt.float32
BF = mybir.dt.bfloat16

B, S, D = 2, 2048, 1024
H, HD = 16, 64
HPC = 4              # heads per core
N_CORES = 8
QSH = S // 4         # per-core q share for out-proj (512)
NT = S // 128        # 16 key/token tiles
NP = 2               # attention q-passes (collective overlap)
PW = QSH // NP       # per-pass width within each q-quarter (256)
PQ = 4 * PW          # per-pass total q columns (1024)
SCALE = HD ** -0.5


def build_nc(use_mask: bool):
    nc = bacc.Bacc("TRN2", target_bir_lowering=False, debug=False,
                   num_devices=N_CORES)

    xt_d = nc.dram_tensor("xt", [D, S], BF, kind="ExternalInput")
    wq_d = nc.dram_tensor("wqT", [D, 256], BF, kind="ExternalInput")
    wk_d = nc.dram_tensor("wkT", [D, 256], BF, kind="ExternalInput")
    wv_d = nc.dram_tensor("wvT", [D, 256], BF, kind="ExternalInput")
    wo_d = nc.dram_tensor("woT", [2 * D, D], BF, kind="ExternalInput")
    cq_d = nc.dram_tensor("ropeQcos", [128, S], BF, kind="ExternalInput")
    sq_d = nc.dram_tensor("ropeQsin", [128, S], BF, kind="ExternalInput")
    ck_d = nc.dram_tensor("ropeKcos", [128, S], BF, kind="ExternalInput")
    sk_d = nc.dram_tensor("ropeKsin", [128, S], BF, kind="ExternalInput")
    if use_mask:
        mb_d = nc.dram_tensor("maskT", [S, S], BF, kind="ExternalInput")
    out_d = nc.dram_tensor("out", [QSH, D], F32, kind="ExternalOutput")

    with tile.TileContext(nc) as tc:
        with tc.tile_pool(name="cpool", bufs=1) as cpool, \
             tc.tile_pool(name="tpool", bufs=2) as tpool, \
             tc.tile_pool(name="ppool", bufs=2, space="PSUM") as ppool, \
             tc.tile_pool(name="dpool", bufs=1, space="DRAM") as dpool:

            def attn_head(hh, p):
                """pass p covers global q cols 512*g + PW*p + [0:PW), g=0..3.
                at_sb columns are pass-local: col = PQ*p + PW*g + j."""
                at_ps = ppool.tile([128, PQ], F32, name="at_ps", tag="at", bufs=2)
                for tt in range(NT):
                    st = ppool.tile([128, PQ], F32, name="st", tag="st", bufs=2)
                    for nck in range(2):
                        nc.tensor.matmul(
                            out=st[:, 512 * nck:512 * nck + 512],
                            lhsT=krp[hh][:, 128 * tt:128 * tt + 128],
                            rhs=qrp[hh][:, PQ * p + 512 * nck:
                                        PQ * p + 512 * nck + 512],
                            start=True, stop=True)
                    if use_mask:
                        mb = tpool.tile([128, PQ], BF, name="mb", tag="mb")
                        nc.sync.dma_start(
                            out=mb[:],
                            in_=mb_d[128 * tt:128 * tt + 128,
                                     PQ * p:PQ * p + PQ])
                        nc.vector.tensor_add(st[:], st[:], mb[:])
                    pt = tpool.tile([128, PQ], BF, name="pt", tag="pt", bufs=4)
                    nc.scalar.activation(out=pt[:], in_=st[:],
                                         func=mybir.ActivationFunctionType.Exp)
                    for nck in range(2):
                        nc.tensor.matmul(
                            out=at_ps[:, 512 * nck:512 * nck + 512],
                            lhsT=vt_all[:, tt, hh, :],
                            rhs=pt[:, 512 * nck:512 * nck + 512],
                            start=(tt == 0), stop=(tt == NT - 1))
                # normalize: r = 1/denom, DRAM-broadcast over 64 partitions
                r65 = tpool.tile([65, PQ], F32, name="r65", tag="r65")
                nc.vector.reciprocal(out=r65[64:65, :], in_=at_ps[64:65, :])
                r_dram = dpool.tile([1, PQ], F32, name="r_dram", tag="r_dram",
                                    bufs=2)
                nc.sync.dma_start(out=r_dram[:], in_=r65[64:65, :])
                rbc = tpool.tile([64, PQ], F32, name="rbc", tag="rbc")
                bcast_src = bass.AP(tensor=r_dram.tensor, offset=r_dram.offset,
                                    ap=[[0, 64]] + [list(q) for q in r_dram.ap[1:]])
                nc.gpsimd.dma_start(out=rbc[:], in_=bcast_src)
                nc.vector.tensor_mul(at_sb[hh][:, PQ * p:PQ * p + PQ],
                                     at_ps[0:64, :], rbc[:])

            def exchange(p, a2a_in, a2a_out):
                for j in range(8):
                    for hh in range(HPC):
                        nc.sync.dma_start(
                            out=a2a_in[256 * j + 64 * hh:
                                       256 * j + 64 * hh + 64, :],
                            in_=at_sb[hh][:, PQ * p + PW * (j % 4):
                                          PQ * p + PW * (j % 4) + PW])
                nc.gpsimd.collective_compute(
                    "AllToAll", mybir.AluOpType.bypass,
                    replica_groups=[list(range(N_CORES))],
                    ins=[a2a_in.opt()], outs=[a2a_out.opt()])

            def outproj(p, a2a_out):
                agt = []
                for kc in range(16):
                    a = wpool.tile([128, PW], BF, name=f"agt{kc}",
                                   tag=f"agt{kc}", bufs=2)
                    nc.sync.dma_start(out=a[:],
                                      in_=a2a_out[128 * kc:128 * kc + 128, :])
                    agt.append(a)
                for qq in range(PW // 128):
                    op = ppool.tile([128, 1024], F32, name="op", tag="st", bufs=2)
                    for nck in range(2):
                        for kc in range(16):
                            nc.tensor.matmul(
                                out=op[:, 512 * nck:512 * nck + 512],
                                lhsT=agt[kc][:, 128 * qq:128 * qq + 128],
                                rhs=wo_sb[kc][:, 512 * nck:512 * nck + 512],
                                start=(kc == 0), stop=(kc == 15))
                    osb = tpool.tile([128, 1024], F32, name="osb", tag="osb")
                    nc.scalar.copy(out=osb[:], in_=op[:])
                    nc.sync.dma_start(
                        out=out_d[PW * p + 128 * qq:PW * p + 128 * qq + 128, :],
                        in_=osb[:])

            def proj_pair(i, w_sb, dsts, ctab, stab, xt_sb):
                """project hd dims [128i:128i+128] for all q, rope, then split
                the head pair into K-padded per-head tiles dsts[0], dsts[1]."""
                raw = tpool.tile([128, S], BF, name="raw", tag="rope_raw",
                                 bufs=1)
                for qh in range(2):
                    ps = ppool.tile([128, 1024], F32, name="proj_ps", tag="st",
                                    bufs=2)
                    for nck in range(2):
                        for kc in range(8):
                            nc.tensor.matmul(
                                out=ps[:, 512 * nck:512 * nck + 512],
                                lhsT=w_sb[kc][:, 128 * i:128 * i + 128],
                                rhs=xt_sb[kc][:, 1024 * qh + 512 * nck:
                                              1024 * qh + 512 * nck + 512],
                                start=(kc == 0), stop=(kc == 7))
                    nc.scalar.copy(out=raw[:, 1024 * qh:1024 * qh + 1024], in_=ps[:])
                shuf = tpool.tile([128, S], BF, name="shuf", tag="rope_shuf",
                                  bufs=1)
                for blk in range(4):
                    src = 32 * (blk ^ 1)
                    nc.sync.dma_start(out=shuf[32 * blk:32 * blk + 32, :],
                                      in_=raw[src:src + 32, :])
                t1 = tpool.tile([128, S], BF, name="t1", tag="rope_t1", bufs=1)
                t2 = tpool.tile([128, S], BF, name="t2", tag="rope_t2", bufs=1)
                nc.vector.tensor_mul(t1[:], raw[:], ctab[:])
                nc.vector.tensor_mul(t2[:], shuf[:], stab[:])
                nc.vector.tensor_add(t1[:], t1[:], t2[:])
                nc.sync.dma_start(out=dsts[0][0:64, :], in_=t1[0:64, :])
                nc.sync.dma_start(out=dsts[1][0:64, :], in_=t1[64:128, :])

            # ---- persistent tiles ----
            # v with ones column at 64 and zero-pad to M=128 (full-array mms)
            vt_all = cpool.tile([128, NT, HPC, 128], BF, name="vt_all",
                                tag="vt_all")
            # per-head K-padded (rows 64..127 zero) rope'd Q/K
            qrp = [cpool.tile([128, S], BF, name=f"qrp{h}", tag=f"qrp{h}")
                   for h in range(HPC)]
            krp = [cpool.tile([128, S], BF, name=f"krp{h}", tag=f"krp{h}")
                   for h in range(HPC)]
            at_sb = [cpool.tile([64, S], BF, name=f"at_sb{h}", tag=f"at_sb{h}")
                     for h in range(HPC)]
            a2a = [(dpool.tile([2 * D, PW], BF, name=f"a2a_in{p}", tag=f"a2a_in{p}"),
                    dpool.tile([2 * D, PW], BF, name=f"a2a_out{p}", tag=f"a2a_out{p}"))
                   for p in range(NP)]

            # ---- phase 1: projections + attention pass 0 ----
            with tc.tile_pool(name="xpool", bufs=1) as xpool:
                xt_sb, wq_sb, wk_sb, wv_sb = [], [], [], []
                for kc in range(8):
                    t = xpool.tile([128, S], BF, name=f"xt{kc}", tag=f"xt{kc}")
                    nc.sync.dma_start(out=t[:], in_=xt_d[128 * kc:128 * kc + 128, :])
                    xt_sb.append(t)
                    for lst, dram, nm in ((wq_sb, wq_d, "wq"),
                                          (wk_sb, wk_d, "wk"),
                                          (wv_sb, wv_d, "wv")):
                        w = xpool.tile([128, 256], BF, name=f"{nm}{kc}",
                                       tag=f"{nm}{kc}")
                        nc.sync.dma_start(out=w[:],
                                          in_=dram[128 * kc:128 * kc + 128, :])
                        lst.append(w)
                tabs = {}
                for nm, dram in (("cq", cq_d), ("sq", sq_d),
                                 ("ck", ck_d), ("sk", sk_d)):
                    t = cpool.tile([128, S], BF, name=f"tab_{nm}", tag=f"tab_{nm}")
                    nc.sync.dma_start(out=t[:], in_=dram[:, :])
                    tabs[nm] = t
                nc.vector.memset(vt_all[:, :, :, 64:65], 1.0)
                nc.vector.memset(vt_all[:, :, :, 65:128], 0.0)
                for h in range(HPC):
                    nc.vector.memset(qrp[h][64:128, :], 0.0)
                    nc.vector.memset(krp[h][64:128, :], 0.0)

                proj_pair(0, wq_sb, (qrp[0], qrp[1]), tabs["cq"], tabs["sq"], xt_sb)
                proj_pair(0, wk_sb, (krp[0], krp[1]), tabs["ck"], tabs["sk"], xt_sb)
                for tt in range(NT):
                    vp = ppool.tile([128, 256], F32, name="vp", tag="st", bufs=2)
                    for kc in range(8):
                        nc.tensor.matmul(out=vp[:],
                                         lhsT=xt_sb[kc][:, 128 * tt:128 * tt + 128],
                                         rhs=wv_sb[kc][:],
                                         start=(kc == 0), stop=(kc == 7))
                    nc.vector.tensor_copy(
                        out=vt_all[:, tt, :, 0:64],
                        in_=vp[:].rearrange("p (h d) -> p h d", h=HPC))

                attn_head(0, 0)
                attn_head(1, 0)
                proj_pair(1, wq_sb, (qrp[2], qrp[3]), tabs["cq"], tabs["sq"], xt_sb)
                proj_pair(1, wk_sb, (krp[2], krp[3]), tabs["ck"], tabs["sk"], xt_sb)

            with tc.tile_pool(name="wpool", bufs=1) as wpool:
                wo_sb = []
                for kc in range(16):
                    w = wpool.tile([128, D], BF, name=f"wo{kc}", tag=f"wo{kc}")
                    nc.sync.dma_start(out=w[:],
                                      in_=wo_d[128 * kc:128 * kc + 128, :])
                    wo_sb.append(w)

                attn_head(2, 0)
                attn_head(3, 0)
                exchange(0, *a2a[0])

                attn_head(0, 1)
                attn_head(1, 1)
                outproj(0, a2a[0][1])
                attn_head(2, 1)
                attn_head(3, 1)
                exchange(1, *a2a[1])
                outproj(1, a2a[1][1])

    nc.compile()
    return nc


_NC_CACHE = {}
TRACE = False
LAST_RESULT = None


def _get_nc(use_mask: bool):
    if use_mask not in _NC_CACHE:
        _NC_CACHE[use_mask] = build_nc(use_mask)
    return _NC_CACHE[use_mask]


def _rope_tables():
    inv_freq = 1.0 / (10000.0 ** (np.arange(0, HD, 2, dtype=np.float32) / HD))
    t = np.arange(S, dtype=np.float32)
    freqs = np.einsum("s,d->sd", t, inv_freq)
    emb = np.concatenate([freqs, freqs], axis=-1)        # (S, 64)
    cos64 = np.cos(emb).T.astype(np.float32)             # (64, S)
    sin64 = np.sin(emb).T.astype(np.float32)
    sign = np.where(np.arange(HD) < 32, -1.0, 1.0)[:, None].astype(np.float32)
    sinp = sign * sin64
    cosT = np.tile(cos64, (2, 1))                        # (128, S)
    sinT = np.tile(sinp, (2, 1))
    return cosT, sinT


def kernel(x, attention_mask, Wq, Wk, Wv, Wo):
    x = np.asarray(x)
    attention_mask = np.asarray(attention_mask)
    Wq, Wk, Wv, Wo = (np.asarray(w) for w in (Wq, Wk, Wv, Wo))

    mask_trivial = bool((attention_mask != 0).all())
    nc = _get_nc(not mask_trivial)

    # pass-local q permutation: device col 1024p+256g+j <- global q 512g+256p+j
    perm = np.empty(S, dtype=np.int64)
    for p in range(NP):
        for g in range(4):
            j = np.arange(PW)
            perm[PQ * p + PW * g + j] = 512 * g + PW * p + j

    cosT, sinT = _rope_tables()
    cosT, sinT = cosT[:, perm], sinT[:, perm]
    cq = (cosT * SCALE).astype(BF16)
    sq = (sinT * SCALE).astype(BF16)
    ck = cosT.astype(BF16)
    sk = sinT.astype(BF16)
    woT = Wo.T.astype(np.float32)                        # (1024, 1024)

    in_maps = []
    for c in range(N_CORES):
        b, hg = c // 4, c % 4
        rows = slice(256 * hg, 256 * hg + 256)
        # per-core zero-padded extended Wo: row 256*i + d is Wo.T row
        # 256*(i%4)+d if sender i is in my batch group, else 0.
        wo_ext = np.zeros((2 * D, D), dtype=BF16)
        my = slice(1024 * b, 1024 * b + 1024)
        wo_ext[my] = woT.astype(BF16)
        m = {
            "xt": np.ascontiguousarray(x[b].T[:, perm]).astype(BF16),
            "wqT": np.ascontiguousarray(Wq[rows, :].T).astype(BF16),
            "wkT": np.ascontiguousarray(Wk[rows, :].T).astype(BF16),
            "wvT": np.ascontiguousarray(Wv[rows, :].T).astype(BF16),
            "woT": wo_ext,
            "ropeQcos": cq, "ropeQsin": sq,
            "ropeKcos": ck, "ropeKsin": sk,
        }
        if not mask_trivial:
            bias = np.where(attention_mask[b] == 0, -30000.0, 0.0)
            m["maskT"] = np.ascontiguousarray(
                bias.T[perm][:, perm]).astype(BF16)
        in_maps.append(m)

    global LAST_RESULT
    res = run_bass_kernel_spmd(nc, in_maps, list(range(N_CORES)),
                               trace=TRACE, trace_cores=list(range(N_CORES)))
    LAST_RESULT = res

    out = np.empty((B, S, D), dtype=np.float32)
    for c in range(N_CORES):
        b, hg = c // 4, c % 4
        out[b, QSH * hg:QSH * hg + QSH, :] = res.results[c]["out"]
    return out


# revision 25
# speedup vs baseline: 1.4414x; 1.1370x over previous
"""MultiHeadAttention with RoPE on 8 Trainium2 NeuronCores.

Sharding: core c = (batch b = c//4, head-group hg = c%4, 4 heads/group).
Per core: bf16 QKV projections (host-pretransposed operands), RoPE via
shuffle-DMA + DVE multiply-adds against host-baked cos/sin tables
(softmax scale folded into Q's tables), flash-style attention in
scores-transposed layout (keys on partitions) with a ones-column appended
to V so the PV matmul also produces softmax denominators, per-head
normalization via fast-reciprocal + DRAM-broadcast, an 8-rank AllToAll
that routes each core the attn^T slice for its q-share (other batch's
shards cancelled by per-core zero-padded Wo), and a q-sharded
out-projection.  Attention runs in two q-passes with one AllToAll per
pass so the collectives overlap compute.  Host reassembles the 8
per-core (512, 1024) outputs.
"""

import sys

sys.path.insert(0, "/opt/trn_rl_repo")

import numpy as np
import ml_dtypes

import concourse.bass as bass
import concourse.bacc as bacc
import concourse.tile as tile
from concourse import mybir
from concourse.bass_utils import run_bass_kernel_spmd

BF16 = ml_dtypes.bfloat16
F32 = mybir.dt.float32
BF = mybir.dt.bfloat16

B, S, D = 2, 2048, 1024
H, HD = 16, 64
HPC = 4              # heads per core
N_CORES = 8
QSH = S // 4         # per-core q share for out-proj (512)
NT = S // 128        # 16 key/token tiles
NP = 2               # attention q-passes (collective overlap)
PW = QSH // NP       # per-pass width within each q-quarter (256)
PQ = 4 * PW          # per-pass total q columns (1024)
SCALE = HD ** -0.5


def build_nc(use_mask: bool):
    nc = bacc.Bacc("TRN2", target_bir_lowering=False, debug=False,
                   num_devices=N_CORES)

    xt_d = nc.dram_tensor("xt", [D, S], BF, kind="ExternalInput")
    wq_d = nc.dram_tensor("wqT", [D, 256], BF, kind="ExternalInput")
    wk_d = nc.dram_tensor("wkT", [D, 256], BF, kind="ExternalInput")
    wv_d = nc.dram_tensor("wvT", [D, 256], BF, kind="ExternalInput")
    wo_d = nc.dram_tensor("woT", [2 * D, D], BF, kind="ExternalInput")
    cq_d = nc.dram_tensor("ropeQcos", [128, S], BF, kind="ExternalInput")
    sq_d = nc.dram_tensor("ropeQsin", [128, S], BF, kind="ExternalInput")
    ck_d = nc.dram_tensor("ropeKcos", [128, S], BF, kind="ExternalInput")
    sk_d = nc.dram_tensor("ropeKsin", [128, S], BF, kind="ExternalInput")
    if use_mask:
        mb_d = nc.dram_tensor("maskT", [S, S], BF, kind="ExternalInput")
    out_d = nc.dram_tensor("out", [QSH, D], F32, kind="ExternalOutput")

    with tile.TileContext(nc) as tc:
        with tc.tile_pool(name="cpool", bufs=1) as cpool, \
             tc.tile_pool(name="tpool", bufs=2) as tpool, \
             tc.tile_pool(name="ppool", bufs=2, space="PSUM") as ppool, \
             tc.tile_pool(name="dpool", bufs=1, space="DRAM") as dpool:

            def attn_head(hh, p):
                """pass p covers global q cols 512*g + PW*p + [0:PW), g=0..3.
                at_sb columns are pass-local: col = PQ*p + PW*g + j."""
                at_ps = ppool.tile([128, PQ], F32, name="at_ps", tag="at", bufs=2)
                for tt in range(NT):
                    st = ppool.tile([128, PQ], F32, name="st", tag="st", bufs=2)
                    for nck in range(2):
                        nc.tensor.matmul(
                            out=st[:, 512 * nck:512 * nck + 512],
                            lhsT=krp[hh][:, 128 * tt:128 * tt + 128],
                            rhs=qrp[hh][:, PQ * p + 512 * nck:
                                        PQ * p + 512 * nck + 512],
                            start=True, stop=True)
                    if use_mask:
                        mb = tpool.tile([128, PQ], BF, name="mb", tag="mb")
                        nc.sync.dma_start(
                            out=mb[:],
                            in_=mb_d[128 * tt:128 * tt + 128,
                                     PQ * p:PQ * p + PQ])
                        nc.vector.tensor_add(st[:], st[:], mb[:])
                    pt = tpool.tile([128, PQ], BF, name="pt", tag="pt", bufs=4)
                    nc.scalar.activation(out=pt[:], in_=st[:],
                                         func=mybir.ActivationFunctionType.Exp)
                    for nck in range(2):
                        nc.tensor.matmul(
                            out=at_ps[:, 512 * nck:512 * nck + 512],
                            lhsT=vt_all[:, tt, hh, :],
                            rhs=pt[:, 512 * nck:512 * nck + 512],
                            start=(tt == 0), stop=(tt == NT - 1))
                # normalize: r = 1/denom, DRAM-broadcast over 64 partitions
                r65 = tpool.tile([65, PQ], F32, name="r65", tag="r65")
                nc.vector.reciprocal(out=r65[64:65, :], in_=at_ps[64:65, :])
                r_dram = dpool.tile([1, PQ], F32, name="r_dram", tag="r_dram",
                                    bufs=2)
                nc.sync.dma_start(out=r_dram[:], in_=r65[64:65, :])
                rbc = tpool.tile([64, PQ], F32, name="rbc", tag="rbc")
                bcast_src = bass.AP(tensor=r_dram.tensor, offset=r_dram.offset,
                                    ap=[[0, 64]] + [list(q) for q in r_dram.ap[1:]])
                nc.gpsimd.dma_start(out=rbc[:], in_=bcast_src)
                nc.vector.tensor_mul(at_sb[hh][:, PQ * p:PQ * p + PQ],
                                     at_ps[0:64, :], rbc[:])

            def exchange(p, a2a_in, a2a_out):
                for j in range(8):
                    for hh in range(HPC):
                        nc.sync.dma_start(
                            out=a2a_in[256 * j + 64 * hh:
                                       256 * j + 64 * hh + 64, :],
                            in_=at_sb[hh][:, PQ * p + PW * (j % 4):
                                          PQ * p + PW * (j % 4) + PW])
                nc.gpsimd.collective_compute(
                    "AllToAll", mybir.AluOpType.bypass,
                    replica_groups=[list(range(N_CORES))],
                    ins=[a2a_in.opt()], outs=[a2a_out.opt()])

            def outproj(p, a2a_out):
                agt = []
                for kc in range(16):
                    a = wpool.tile([128, PW], BF, name=f"agt{kc}",
                                   tag=f"agt{kc}", bufs=2)
                    nc.sync.dma_start(out=a[:],
                                      in_=a2a_out[128 * kc:128 * kc + 128, :])
                    agt.append(a)
                for qq in range(PW // 128):
                    op = ppool.tile([128, 1024], F32, name="op", tag="st", bufs=2)
                    for nck in range(2):
                        for kc in range(16):
                            nc.tensor.matmul(
                                out=op[:, 512 * nck:512 * nck + 512],
                                lhsT=agt[kc][:, 128 * qq:128 * qq + 128],
                                rhs=wo_sb[kc][:, 512 * nck:512 * nck + 512],
                                start=(kc == 0), stop=(kc == 15))
                    osb = tpool.tile([128, 1024], F32, name="osb", tag="osb")
                    nc.scalar.copy(out=osb[:], in_=op[:])
                    nc.sync.dma_start(
                        out=out_d[PW * p + 128 * qq:PW * p + 128 * qq + 128, :],
                        in_=osb[:])

            def proj_pair(i, w_sb, dsts, ctab, stab, xt_sb):
                """project hd dims [128i:128i+128] for all q, rope, then split
                the head pair into K-padded per-head tiles dsts[0], dsts[1]."""
                raw = tpool.tile([128, S], BF, name="raw", tag="rope_raw",
                                 bufs=1)
                for qh in range(2):
                    ps = ppool.tile([128, 1024], F32, name="proj_ps", tag="st",
                                    bufs=2)
                    for nck in range(2):
                        for kc in range(8):
                            nc.tensor.matmul(
                                out=ps[:, 512 * nck:512 * nck + 512],
                                lhsT=w_sb[kc][:, 128 * i:128 * i + 128],
                                rhs=xt_sb[kc][:, 1024 * qh + 512 * nck:
                                              1024 * qh + 512 * nck + 512],
                                start=(kc == 0), stop=(kc == 7))
                    nc.scalar.copy(out=raw[:, 1024 * qh:1024 * qh + 1024], in_=ps[:])
                shuf = tpool.tile([128, S], BF, name="shuf", tag="rope_shuf",
                                  bufs=1)
                for blk in range(4):
                    src = 32 * (blk ^ 1)
                    nc.sync.dma_start(out=shuf[32 * blk:32 * blk + 32, :],
                                      in_=raw[src:src + 32, :])
                t1 = tpool.tile([128, S], BF, name="t1", tag="rope_t1", bufs=1)
                t2 = tpool.tile([128, S], BF, name="t2", tag="rope_t2", bufs=1)
                nc.vector.tensor_mul(t1[:], raw[:], ctab[:])
                nc.vector.tensor_mul(t2[:], shuf[:], stab[:])
                nc.vector.tensor_add(t1[:], t1[:], t2[:])
                nc.sync.dma_start(out=dsts[0][0:64, :], in_=t1[0:64, :])
                nc.sync.dma_start(out=dsts[1][0:64, :], in_=t1[64:128, :])

            # ---- persistent tiles ----
            # v with ones column at 64 and zero-pad to M=128 (full-array mms)
            vt_all = cpool.tile([128, NT, HPC, 128], BF, name="vt_all",
                                tag="vt_all")
            # per-head K-padded (rows 64..127 zero) rope'd Q/K
            qrp = [cpool.tile([128, S], BF, name=f"qrp{h}", tag=f"qrp{h}")
                   for h in range(HPC)]
            krp = [cpool.tile([128, S], BF, name=f"krp{h}", tag=f"krp{h}")
                   for h in range(HPC)]
            at_sb = [cpool.tile([64, S], BF, name=f"at_sb{h}", tag=f"at_sb{h}")
                     for h in range(HPC)]
            a2a = [(dpool.tile([2 * D, PW], BF, name=f"a2a_in{p}", tag=f"a2a_in{p}"),
                    dpool.tile([2 * D, PW], BF, name=f"a2a_out{p}", tag=f"a2a_out{p}"))
                   for p in range(NP)]

            # ---- phase 1: projections + attention pass 0 ----
            with tc.tile_pool(name="xpool", bufs=1) as xpool:
                xt_sb, wq_sb, wk_sb, wv_sb = [], [], [], []
                for kc in range(8):
                    t = xpool.tile([128, S], BF, name=f"xt{kc}", tag=f"xt{kc}")
                    nc.sync.dma_start(out=t[:], in_=xt_d[128 * kc:128 * kc + 128, :])
                    xt_sb.append(t)
                    for lst, dram, nm in ((wq_sb, wq_d, "wq"),
                                          (wk_sb, wk_d, "wk"),
                                          (wv_sb, wv_d, "wv")):
                        w = xpool.tile([128, 256], BF, name=f"{nm}{kc}",
                                       tag=f"{nm}{kc}")
                        nc.sync.dma_start(out=w[:],
                                          in_=dram[128 * kc:128 * kc + 128, :])
                        lst.append(w)
                tabs = {}
                for nm, dram in (("cq", cq_d), ("sq", sq_d),
                                 ("ck", ck_d), ("sk", sk_d)):
                    t = cpool.tile([128, S], BF, name=f"tab_{nm}", tag=f"tab_{nm}")
                    nc.sync.dma_start(out=t[:], in_=dram[:, :])
                    tabs[nm] = t
                nc.vector.memset(vt_all[:, :, :, 64:65], 1.0)
                nc.vector.memset(vt_all[:, :, :, 65:128], 0.0)
                for h in range(HPC):
                    nc.vector.memset(qrp[h][64:128, :], 0.0)
                    nc.vector.memset(krp[h][64:128, :], 0.0)

                proj_pair(0, wq_sb, (qrp[0], qrp[1]), tabs["cq"], tabs["sq"], xt_sb)
                proj_pair(0, wk_sb, (krp[0], krp[1]), tabs["ck"], tabs["sk"], xt_sb)
                for tt in range(NT):
                    vp = ppool.tile([128, 256], F32, name="vp", tag="st", bufs=2)
                    for kc in range(8):
                        nc.tensor.matmul(out=vp[:],
                                         lhsT=xt_sb[kc][:, 128 * tt:128 * tt + 128],
                                         rhs=wv_sb[kc][:],
                                         start=(kc == 0), stop=(kc == 7))
                    nc.vector.tensor_copy(
                        out=vt_all[:, tt, :, 0:64],
                        in_=vp[:].rearrange("p (h d) -> p h d", h=HPC))

                attn_head(0, 0)
                proj_pair(1, wq_sb, (qrp[2], qrp[3]), tabs["cq"], tabs["sq"], xt_sb)
                proj_pair(1, wk_sb, (krp[2], krp[3]), tabs["ck"], tabs["sk"], xt_sb)
                attn_head(1, 0)

            with tc.tile_pool(name="wpool", bufs=1) as wpool:
                wo_sb = []
                for kc in range(16):
                    w = wpool.tile([128, D], BF, name=f"wo{kc}", tag=f"wo{kc}")
                    nc.sync.dma_start(out=w[:],
                                      in_=wo_d[128 * kc:128 * kc + 128, :])
                    wo_sb.append(w)

                attn_head(2, 0)
                attn_head(3, 0)
                exchange(0, *a2a[0])

                attn_head(0, 1)
                attn_head(1, 1)
                attn_head(2, 1)
                attn_head(3, 1)
                exchange(1, *a2a[1])
                outproj(0, a2a[0][1])
                outproj(1, a2a[1][1])

    nc.compile()
    return nc


_NC_CACHE = {}
TRACE = False
LAST_RESULT = None


def _get_nc(use_mask: bool):
    if use_mask not in _NC_CACHE:
        _NC_CACHE[use_mask] = build_nc(use_mask)
    return _NC_CACHE[use_mask]


def _rope_tables():
    inv_freq = 1.0 / (10000.0 ** (np.arange(0, HD, 2, dtype=np.float32) / HD))
    t = np.arange(S, dtype=np.float32)
    freqs = np.einsum("s,d->sd", t, inv_freq)
    emb = np.concatenate([freqs, freqs], axis=-1)        # (S, 64)
    cos64 = np.cos(emb).T.astype(np.float32)             # (64, S)
    sin64 = np.sin(emb).T.astype(np.float32)
    sign = np.where(np.arange(HD) < 32, -1.0, 1.0)[:, None].astype(np.float32)
    sinp = sign * sin64
    cosT = np.tile(cos64, (2, 1))                        # (128, S)
    sinT = np.tile(sinp, (2, 1))
    return cosT, sinT


def kernel(x, attention_mask, Wq, Wk, Wv, Wo):
    x = np.asarray(x)
    attention_mask = np.asarray(attention_mask)
    Wq, Wk, Wv, Wo = (np.asarray(w) for w in (Wq, Wk, Wv, Wo))

    mask_trivial = bool((attention_mask != 0).all())
    nc = _get_nc(not mask_trivial)

    # pass-local q permutation: device col 1024p+256g+j <- global q 512g+256p+j
    perm = np.empty(S, dtype=np.int64)
    for p in range(NP):
        for g in range(4):
            j = np.arange(PW)
            perm[PQ * p + PW * g + j] = 512 * g + PW * p + j

    cosT, sinT = _rope_tables()
    cosT, sinT = cosT[:, perm], sinT[:, perm]
    cq = (cosT * SCALE).astype(BF16)
    sq = (sinT * SCALE).astype(BF16)
    ck = cosT.astype(BF16)
    sk = sinT.astype(BF16)
    woT = Wo.T.astype(np.float32)                        # (1024, 1024)

    in_maps = []
    for c in range(N_CORES):
        b, hg = c // 4, c % 4
        rows = slice(256 * hg, 256 * hg + 256)
        # per-core zero-padded extended Wo: row 256*i + d is Wo.T row
        # 256*(i%4)+d if sender i is in my batch group, else 0.
        wo_ext = np.zeros((2 * D, D), dtype=BF16)
        my = slice(1024 * b, 1024 * b + 1024)
        wo_ext[my] = woT.astype(BF16)
        m = {
            "xt": np.ascontiguousarray(x[b].T[:, perm]).astype(BF16),
            "wqT": np.ascontiguousarray(Wq[rows, :].T).astype(BF16),
            "wkT": np.ascontiguousarray(Wk[rows, :].T).astype(BF16),
            "wvT": np.ascontiguousarray(Wv[rows, :].T).astype(BF16),
            "woT": wo_ext,
            "ropeQcos": cq, "ropeQsin": sq,
            "ropeKcos": ck, "ropeKsin": sk,
        }
        if not mask_trivial:
            bias = np.where(attention_mask[b] == 0, -30000.0, 0.0)
            m["maskT"] = np.ascontiguousarray(
                bias.T[perm][:, perm]).astype(BF16)
        in_maps.append(m)

    global LAST_RESULT
    res = run_bass_kernel_spmd(nc, in_maps, list(range(N_CORES)),
                               trace=TRACE, trace_cores=list(range(N_CORES)))
    LAST_RESULT = res

    out = np.empty((B, S, D), dtype=np.float32)
    for c in range(N_CORES):
        b, hg = c // 4, c % 4
        out[b, QSH * hg:QSH * hg + QSH, :] = res.results[c]["out"]
    return out
